# revision 1
# baseline (speedup 1.0000x reference)
"""Grouped per-expert SwiGLU FFN (MoE experts) on 8 TRN2 NeuronCores.

Reference computation (per expert e):
    h1  = x[e] @ w1[e]          # [T, H]
    h3  = x[e] @ w3[e]          # [T, H]
    mid = silu(h1) * h3         # [T, H]
    out = mid @ w2[e].T         # [T, D]

Sharding: expert-parallel, one expert per core (E == n_cores == 8).
No collectives needed; outputs concatenate along E.

Per-core kernel (T=256, D=2048, H=8192), bf16 compute / fp32 accumulate:
  Host pre-processing per expert: xt = x[e].T (D,T) bf16, w1/w3 (D,H) bf16,
  w2t = w2[e].T (H,D) bf16. All natural-layout DMA loads, no on-chip
  transposes anywhere. Weight DMA is split across both HWDGE rings
  (SP: w1 + half of each w2t slice; ACT: xt + w3 + other half) -- a single
  ring serializes the loads against compute and costs ~1.6x end to end:
    Phase 1 (up/gate): for each 128-wide slice of H, accumulate over D
      h1T/h3T [128h, 256t] in PSUM (lhsT = w1/w3 tile [128d, 128h],
      rhs = xt tile [128d, 256t]), then midT[:, ht, :] = silu(h1T) * h3T
      (bf16, kept in SBUF: [128h, 64ht, 256t]).
    Phase 2 (down): for each 512-wide slice of D, accumulate over all of H
      out[t-tile, d-slice] (lhsT = midT sub-tile [128h, 128t],
      rhs = w2t tile [128h, 512d]) over 64 h-tiles in PSUM, copy to SBUF,
      stream each [128, 512] fp32 chunk straight to DRAM.
"""

import sys

if "/opt/trn_rl_repo" not in sys.path:
    sys.path.insert(0, "/opt/trn_rl_repo")

import numpy as np
import ml_dtypes

import concourse.mybir as mybir
import concourse.tile as tile
from concourse import bacc
from concourse.bass_utils import run_bass_kernel_spmd

E, T, D, H = 8, 256, 2048, 8192
P = 128
KD = D // P          # 16 d-tiles (contraction tiles for up/gate proj)
HT = H // P          # 64 h-tiles
TT = T // P          # 2 t-tiles
H_SLICE = 512        # w1/w3 DMA slice width along H (1 KiB rows)
HS = H // H_SLICE    # 16 slices
D_SLICE = 512
DS = D // D_SLICE    # 4 slices

BF16 = mybir.dt.bfloat16
F32 = mybir.dt.float32

_CACHED = {}


def _build(reps: int = 1):
    nc = bacc.Bacc("TRN2", target_bir_lowering=False, debug=False)
    xt_d = nc.dram_tensor("xt", [D, T], BF16, kind="ExternalInput").ap()
    w1_d = nc.dram_tensor("w1", [D, H], BF16, kind="ExternalInput").ap()
    w3_d = nc.dram_tensor("w3", [D, H], BF16, kind="ExternalInput").ap()
    w2t_d = nc.dram_tensor("w2t", [H, D], BF16, kind="ExternalInput").ap()
    out_d = nc.dram_tensor("out", [T, D], F32, kind="ExternalOutput").ap()

    # [ (o p) f -> p o f ] views put 128 consecutive rows on partitions.
    xt_v = xt_d.rearrange("(o p) t -> p o t", p=P)      # [128, 16, 256]
    w1_v = w1_d.rearrange("(o p) h -> p o h", p=P)      # [128, 16, 8192]
    w3_v = w3_d.rearrange("(o p) h -> p o h", p=P)      # [128, 16, 8192]
    w2t_v = w2t_d.rearrange("(o p) d -> p o d", p=P)    # [128, 64, 2048]
    out_v = out_d.rearrange("(o p) d -> p o d", p=P)    # [128, 2, 2048]

    with tile.TileContext(nc) as tc:
        with tc.tile_pool(name="persist", bufs=1) as cpool:
          for _rep in range(reps):
            xt_sb = cpool.tile([P, KD, T], BF16, tag="xt", name="xt_sb")
            midT = cpool.tile([P, HT, T], BF16, tag="midT", name="midT")

            nc.scalar.dma_start(xt_sb, xt_v)

            # ---- Phase 1: up/gate projections + SwiGLU -> midT ----
            with (
                tc.tile_pool(name="wpool", bufs=4) as wpool,
                tc.tile_pool(name="act", bufs=3) as apool,
                tc.tile_pool(name="ps1", bufs=3, space="PSUM") as ps1,
            ):
                for j in range(HS):
                    w1_sb = wpool.tile([P, KD, H_SLICE], BF16, tag="w1", name="w1_sb")
                    w3_sb = wpool.tile([P, KD, H_SLICE], BF16, tag="w3", name="w3_sb")
                    hsl = slice(j * H_SLICE, (j + 1) * H_SLICE)
                    nc.sync.dma_start(w1_sb, w1_v[:, :, hsl])
                    nc.scalar.dma_start(w3_sb, w3_v[:, :, hsl])
                    for s in range(H_SLICE // P):
                        ht = j * (H_SLICE // P) + s
                        ssl = slice(s * P, (s + 1) * P)
                        h1_ps = ps1.tile([P, T], F32, tag="h1", name="h1_ps")
                        h3_ps = ps1.tile([P, T], F32, tag="h3", name="h3_ps")
                        for kd in range(KD):
                            nc.tensor.matmul(
                                h1_ps,
                                w1_sb[:, kd, ssl],
                                xt_sb[:, kd, :],
                                start=(kd == 0),
                                stop=(kd == KD - 1),
                            )
                        for kd in range(KD):
                            nc.tensor.matmul(
                                h3_ps,
                                w3_sb[:, kd, ssl],
                                xt_sb[:, kd, :],
                                start=(kd == 0),
                                stop=(kd == KD - 1),
                            )
                        silu_sb = apool.tile([P, T], F32, tag="silu", name="silu_sb")
                        nc.scalar.activation(
                            silu_sb, h1_ps, mybir.ActivationFunctionType.Silu
                        )
                        nc.vector.tensor_mul(
                            out=midT[:, ht, :], in0=silu_sb, in1=h3_ps
                        )

            # ---- Phase 2: down projection (ht-outer, 8 psum banks) ----
            with (
                tc.tile_pool(name="w2pool", bufs=6) as w2pool,
                tc.tile_pool(name="opool", bufs=3) as opool,
                tc.tile_pool(name="ps2", bufs=1, space="PSUM") as ps2,
            ):
                o_ps = [
                    [
                        ps2.tile([P, D_SLICE], F32, tag=f"o{t}_{dsl}",
                                 name=f"o_ps_{t}_{dsl}")
                        for dsl in range(DS)
                    ]
                    for t in range(TT)
                ]
                HCH = 4  # h-tiles per DMA chunk (2 MiB)
                for hc in range(HT // HCH):
                    w2_sb = w2pool.tile([P, HCH, D], BF16, tag="w2",
                                        name="w2_sb")
                    eng = nc.sync if hc % 2 == 0 else nc.scalar
                    eng.dma_start(w2_sb, w2t_v[:, hc * HCH:(hc + 1) * HCH, :])
                    for hh in range(HCH):
                        ht = hc * HCH + hh
                        for t in range(TT):
                            tsl = slice(t * P, (t + 1) * P)
                            for dsl in range(DS):
                                nc.tensor.matmul(
                                    o_ps[t][dsl],
                                    midT[:, ht, tsl],
                                    w2_sb[:, hh, dsl * D_SLICE:(dsl + 1) * D_SLICE],
                                    start=(ht == 0),
                                    stop=(ht == HT - 1),
                                )
                for t in range(TT):
                    for dsl in range(DS):
                        dslice = slice(dsl * D_SLICE, (dsl + 1) * D_SLICE)
                        o_sb = opool.tile([P, D_SLICE], F32, tag="osb",
                                          name="o_sb")
                        nc.any.tensor_copy(out=o_sb, in_=o_ps[t][dsl])
                        nc.sync.dma_start(out_v[:, t, dslice], o_sb)

    nc.compile()
    return nc


def _get_nc():
    if "nc" not in _CACHED:
        _CACHED["nc"] = _build()
    return _CACHED["nc"]


def kernel(x, w1, w2, w3, **_unused):
    """x: [E,T,D] f32; w1,w2,w3: [E,D,H] f32 -> [E,T,D] f32."""
    bf = ml_dtypes.bfloat16
    in_maps = []
    for e in range(E):
        in_maps.append(
            {
                "xt": np.ascontiguousarray(np.asarray(x[e]).T).astype(bf),
                "w1": np.asarray(w1[e]).astype(bf),
                "w3": np.asarray(w3[e]).astype(bf),
                "w2t": np.ascontiguousarray(np.asarray(w2[e]).T).astype(bf),
            }
        )
    nc = _get_nc()
    res = run_bass_kernel_spmd(nc, in_maps, core_ids=list(range(E)))
    out = np.stack([res.results[e]["out"] for e in range(E)], axis=0)
    return out.astype(np.float32, copy=False)



# revision 2
# speedup vs baseline: 1.5899x; 1.5899x over previous
"""Grouped per-expert SwiGLU FFN (MoE experts) on 8 TRN2 NeuronCores. v2.

Same dataflow as v1 (one expert per core, bf16, all-natural-layout DMA):
  Phase 1: h1T/h3T accumulated in PSUM per 128-wide h-slice; midT in SBUF.
  Phase 2: out accumulation over all h in 8 PSUM banks.

v2 changes (driven by microbenchmarks: PE hits its 109.2us/262k-row
roofline even with per-matmul LDWEIGHTS; 2-ring DMA sustains ~530 GB/s):
  - w2t pool is open CONCURRENTLY with the phase-1 weight pool and its
    chunk DMAs are interleaved into the back half of phase 1's program
    order, so phase-2 weights are buffered before phase 1 drains (the v1
    phase boundary serialized w2 loads behind the phase-1 pool teardown).
  - wpool bufs=3 / w2pool bufs=4 sized to fit SBUF alongside xt+midT.
"""

import sys

if "/opt/trn_rl_repo" not in sys.path:
    sys.path.insert(0, "/opt/trn_rl_repo")

import numpy as np
import ml_dtypes

import concourse.mybir as mybir
import concourse.tile as tile
from concourse import bacc
from concourse.bass_utils import run_bass_kernel_spmd

E, T, D, H = 8, 256, 2048, 8192
P = 128
KD = D // P          # 16 d-tiles
HT = H // P          # 64 h-tiles
TT = T // P          # 2 t-tiles
H_SLICE = 512
HS = H // H_SLICE    # 16 slices
D_SLICE = 512
DS = D // D_SLICE    # 4 slices
HCH = 4              # w2t h-tiles per DMA chunk (2 MiB)
NCH = HT // HCH      # 16 chunks

BF16 = mybir.dt.bfloat16
F32 = mybir.dt.float32

_CACHED = {}


def _build(reps: int = 1):
    nc = bacc.Bacc("TRN2", target_bir_lowering=False, debug=False)
    xt_d = nc.dram_tensor("xt", [D, T], BF16, kind="ExternalInput").ap()
    w1_d = nc.dram_tensor("w1", [D, H], BF16, kind="ExternalInput").ap()
    w3_d = nc.dram_tensor("w3", [D, H], BF16, kind="ExternalInput").ap()
    w2t_d = nc.dram_tensor("w2t", [H, D], BF16, kind="ExternalInput").ap()
    out_d = nc.dram_tensor("out", [T, D], F32, kind="ExternalOutput").ap()

    xt_v = xt_d.rearrange("(o p) t -> p o t", p=P)      # [128, 16, 256]
    w1_v = w1_d.rearrange("(o p) h -> p o h", p=P)      # [128, 16, 8192]
    w3_v = w3_d.rearrange("(o p) h -> p o h", p=P)      # [128, 16, 8192]
    w2t_v = w2t_d.rearrange("(o p) d -> p o d", p=P)    # [128, 64, 2048]
    out_v = out_d.rearrange("(o p) d -> p o d", p=P)    # [128, 2, 2048]

    with tile.TileContext(nc) as tc:
        with tc.tile_pool(name="persist", bufs=1) as cpool:
          for _rep in range(reps):
            xt_sb = cpool.tile([P, KD, T], BF16, tag="xt", name="xt_sb")
            midT = cpool.tile([P, HT, T], BF16, tag="midT", name="midT")

            nc.scalar.dma_start(xt_sb, xt_v)

            with (
                tc.tile_pool(name="wpool", bufs=3) as wpool,
                tc.tile_pool(name="w2pool", bufs=3) as w2pool,
                tc.tile_pool(name="act", bufs=3) as apool,
                tc.tile_pool(name="opool", bufs=3) as opool,
            ):
                w2_tiles = []

                def w2_load(c):
                    w2_sb = w2pool.tile([P, HCH, D], BF16, tag="w2",
                                        name="w2_sb")
                    eng = nc.sync if c % 2 == 0 else nc.scalar
                    eng.dma_start(w2_sb, w2t_v[:, c * HCH:(c + 1) * HCH, :])
                    w2_tiles.append(w2_sb)

                # ---- Phase 1: up/gate projections + SwiGLU -> midT ----
                with tc.tile_pool(name="ps1", bufs=3, space="PSUM") as ps1:
                    for j in range(HS):
                        w1_sb = wpool.tile([P, KD, H_SLICE], BF16, tag="w1",
                                           name="w1_sb")
                        w3_sb = wpool.tile([P, KD, H_SLICE], BF16, tag="w3",
                                           name="w3_sb")
                        hsl = slice(j * H_SLICE, (j + 1) * H_SLICE)
                        nc.sync.dma_start(w1_sb, w1_v[:, :, hsl])
                        nc.scalar.dma_start(w3_sb, w3_v[:, :, hsl])
                        if j >= 13:  # prefetch first w2t chunks late in phase 1
                            w2_load(j - 13)
                        for s in range(H_SLICE // P):
                            ht = j * (H_SLICE // P) + s
                            ssl = slice(s * P, (s + 1) * P)
                            h1_ps = ps1.tile([P, T], F32, tag="h1", name="h1_ps")
                            h3_ps = ps1.tile([P, T], F32, tag="h3", name="h3_ps")
                            for kd in range(KD):
                                nc.tensor.matmul(
                                    h1_ps,
                                    w1_sb[:, kd, ssl],
                                    xt_sb[:, kd, :],
                                    start=(kd == 0),
                                    stop=(kd == KD - 1),
                                )
                            for kd in range(KD):
                                nc.tensor.matmul(
                                    h3_ps,
                                    w3_sb[:, kd, ssl],
                                    xt_sb[:, kd, :],
                                    start=(kd == 0),
                                    stop=(kd == KD - 1),
                                )
                            silu_sb = apool.tile([P, T], F32, tag="silu",
                                                 name="silu_sb")
                            nc.scalar.activation(
                                silu_sb, h1_ps, mybir.ActivationFunctionType.Silu
                            )
                            nc.vector.tensor_mul(
                                out=midT[:, ht, :], in0=silu_sb, in1=h3_ps
                            )

                # ---- Phase 2: down projection (ht-outer, 8 psum banks) ----
                with tc.tile_pool(name="ps2", bufs=1, space="PSUM") as ps2:
                    o_ps = [
                        [
                            ps2.tile([P, D_SLICE], F32, tag=f"o{t}_{dsl}",
                                     name=f"o_ps_{t}_{dsl}")
                            for dsl in range(DS)
                        ]
                        for t in range(TT)
                    ]
                    for hc in range(NCH):
                        if hc + 3 < NCH:
                            w2_load(hc + 3)
                        w2_sb = w2_tiles[hc]
                        for hh in range(HCH):
                            ht = hc * HCH + hh
                            for t in range(TT):
                                tsl = slice(t * P, (t + 1) * P)
                                for dsl in range(DS):
                                    nc.tensor.matmul(
                                        o_ps[t][dsl],
                                        midT[:, ht, tsl],
                                        w2_sb[:, hh,
                                              dsl * D_SLICE:(dsl + 1) * D_SLICE],
                                        start=(ht == 0),
                                        stop=(ht == HT - 1),
                                    )
                    for t in range(TT):
                        for dsl in range(DS):
                            dslice = slice(dsl * D_SLICE, (dsl + 1) * D_SLICE)
                            o_sb = opool.tile([P, D_SLICE], F32, tag="osb",
                                              name="o_sb")
                            nc.any.tensor_copy(out=o_sb, in_=o_ps[t][dsl])
                            (nc.sync if dsl % 2 == 0 else nc.scalar).dma_start(
                                out_v[:, t, dslice], o_sb)

    nc.compile()
    return nc


def _get_nc():
    if "nc" not in _CACHED:
        _CACHED["nc"] = _build()
    return _CACHED["nc"]


def kernel(x, w1, w2, w3, **_unused):
    """x: [E,T,D] f32; w1,w2,w3: [E,D,H] f32 -> [E,T,D] f32."""
    bf = ml_dtypes.bfloat16
    in_maps = []
    for e in range(E):
        in_maps.append(
            {
                "xt": np.ascontiguousarray(np.asarray(x[e]).T).astype(bf),
                "w1": np.asarray(w1[e]).astype(bf),
                "w3": np.asarray(w3[e]).astype(bf),
                "w2t": np.ascontiguousarray(np.asarray(w2[e]).T).astype(bf),
            }
        )
    nc = _get_nc()
    res = run_bass_kernel_spmd(nc, in_maps, core_ids=list(range(E)))
    out = np.stack([res.results[e]["out"] for e in range(E)], axis=0)
    return out.astype(np.float32, copy=False)
